# revision 16
# baseline (speedup 1.0000x reference)
"""Trainium2 Bass kernel for nn_DWTExtractor: 2-level Haar DWT + bilinear 2x upsample.

Input  x: (32, 1, 1024, 1024) fp32
Output y: (32, 6, 512, 512) fp32 = [cH1, cV1, cD1, cH2u, cV2u, cD2u]

Sharding: pure batch data-parallel, 4 images per core across 8 cores.

v8 design (PE-minimal: every input column enters the PE exactly once).
The chip power-throttles with all 8 cores running (~55-75% duty), so PE
columns are the scarcest resource; vector-engine combines are cheaper
than extra matmul passes.
  - fp16 datapath (host converts, ~1e-3 rel err).
  - L1 Haar per 128-row block: A = WF @ x_even, B = WF @ x_odd (fused
    [row-pair sums | diffs] weight, strided rhs; 2 matmuls per block =
    PE-optimal). Evac splits A to ACT / B to DVE; U = A+B = [cA1|cV1],
    V = A-B = [cH1|cD1] as packed-fp16 tensor_tensor (2x mode) on
    DVE/GPSIMD into [128, 4096] staging; one output DMA per L1 band.
  - L2 on cA1 (Ustg parts 0..63, zero-padded weights): 2 contiguous-rhs
    matmuls per 128-row group -> parity-interleaved psum; ACT copies it
    out; strided fp16 combines place cH2 LANE-ALIGNED into row-major
    b3all (per-parity weight variants put S2 at partitions 64(g%2)),
    cV2/cD2 into VDtmp + 4 uniform-shift DMAs.
  - W-upsample: t3 = 3*b3 (tensor_scalar 4x) + two shifted adds (2x)
    into parity-BLOCKED wall; e/o interleave deferred to H-up evac APs.
  - H-upsample: 12 matmuls + halo row swap.
  - Hazards: PSUM groups that interleave must sit in different banks;
    DMA dst APs need one uniform partition shift; >3-dim DMA APs fail.
  - Triggers: Sync = input + VD shifts + halo (HWDGE), GPSIMD = band
    outputs (SWDGE); GPSIMD also takes 3 blocks' L1 combines.
"""

import numpy as np

import concourse.bass as bass
import concourse.tile as tile
import concourse.mybir as mybir
from concourse import bacc, bass_utils

F32 = mybir.dt.float32
F16 = mybir.dt.float16
AL = mybir.AluOpType

B, H, W = 32, 1024, 1024
NCORES = 8
IMG = B // NCORES  # images per core
HL, WL = H // 2, W // 2  # 512 (level-1 band size)
H2, W2 = H // 4, W // 4  # 256 (level-2 band size)
P = 128


def _build_w16() -> np.ndarray:
    """(128, 9*128) fp16: WF | A0 B0 A1 B1 | U0 U1p U2p U3.

    WF: out parts [row-pair sums | row-pair diffs].  A/B (parity q):
    S2-pairs at out parts 64q+i, D2-pairs at 64(1-q)+i.
    """
    wf = np.zeros((P, P), np.float16)
    for i in range(64):
        wf[2 * i, i] = 0.5
        wf[2 * i + 1, i] = 0.5
        wf[2 * i, 64 + i] = 0.5
        wf[2 * i + 1, 64 + i] = -0.5

    wl2 = []
    for q in (0, 1):
        a = np.zeros((P, P), np.float16)
        bq = np.zeros((P, P), np.float16)
        so, do = 64 * q, 64 * (1 - q)
        for i in range(32):
            a[2 * i, so + i] = 0.5
            a[2 * i + 1, so + i] = 0.5
            a[2 * i, do + i] = 0.5
            a[2 * i + 1, do + i] = -0.5
            bq[2 * i, so + 32 + i] = 0.5
            bq[2 * i + 1, so + 32 + i] = 0.5
            bq[2 * i, do + 32 + i] = 0.5
            bq[2 * i + 1, do + 32 + i] = -0.5
        wl2 += [a, bq]

    u_full = np.zeros((H2, HL), np.float32)
    for m in range(HL):
        k = m // 2
        taps = [(k, 0.75), (k - 1, 0.25)] if m % 2 == 0 else [(k, 0.75), (k + 1, 0.25)]
        for src, wgt in taps:
            u_full[min(max(src, 0), H2 - 1), m] += wgt
    u_full *= 0.25
    u0 = u_full[0:128, 0:128].astype(np.float16)
    u1p = u_full[0:128, 128:256].astype(np.float16)
    u1p[0, :] = u_full[128, 128:256].astype(np.float16)  # halo tap row
    u2p = u_full[128:256, 256:384].astype(np.float16)
    u2p[127, :] = u_full[127, 256:384].astype(np.float16)  # halo tap row
    u3 = u_full[128:256, 384:512].astype(np.float16)

    return np.concatenate([wf] + wl2 + [u0, u1p, u2p, u3], axis=1)


def build_nc() -> "bacc.Bacc":
    nc = bacc.Bacc(
        "TRN2", target_bir_lowering=False, debug=False, num_devices=NCORES,
        name="dwt_extractor",
    )
    x_d = nc.dram_tensor("xc", [IMG, H, W], F16, kind="ExternalInput")
    w16_d = nc.dram_tensor("w16", [P, 9 * P], F16, kind="ExternalInput")
    y_d = nc.dram_tensor("yc", [IMG, 6, HL, WL], F16, kind="ExternalOutput")

    with tile.TileContext(nc) as tc:
        with (
            tc.tile_pool(name="consts", bufs=1) as cpool,
            tc.tile_pool(name="xin", bufs=3) as xpool,
            tc.tile_pool(name="ab", bufs=3) as abpool,
            tc.tile_pool(name="uv", bufs=2) as uvpool,
            tc.tile_pool(name="l2", bufs=2) as l2pool,
            tc.tile_pool(name="vdt", bufs=2) as vdpool,
            tc.tile_pool(name="b3", bufs=2) as b3pool,
            tc.tile_pool(name="t3p", bufs=2) as t3pool,
            tc.tile_pool(name="wtile", bufs=2) as wpool,
            tc.tile_pool(name="stg2", bufs=2) as stpool,
            tc.tile_pool(name="psL1", bufs=4, space="PSUM") as psL1,
            tc.tile_pool(name="psL2", bufs=2, space="PSUM") as psL2,
            tc.tile_pool(name="psUp", bufs=2, space="PSUM") as psUp,
        ):
            w16 = cpool.tile([P, 9 * P], F16)
            nc.sync.dma_start(w16[:], w16_d[:])
            blk = lambda i: w16[:, i * P : (i + 1) * P]
            WF = blk(0)
            WL2 = [(blk(1), blk(2)), (blk(3), blk(4))]  # [q] -> (A, B)
            U0, U1p, U2p, U3 = blk(5), blk(6), blk(7), blk(8)

            def l1_half(b, hf, Ustg, Vstg):
                """Four 128-row blocks: one load; per block A/B matmuls,
                split evac (ACT/DVE), packed-fp16 combines."""
                xu = xpool.tile([P, 4096], F16, tag="x")
                src = x_d[b, 512 * hf : 512 * (hf + 1), :]
                nc.sync.dma_start(
                    xu[:].rearrange("p (t w) -> p t w", t=4),
                    src.rearrange("(t p) w -> p t w", t=4),
                )
                for t in range(4):
                    u = 4 * hf + t
                    xb = xu[:, 1024 * t : 1024 * (t + 1)]
                    psA = psL1.tile([P, 512], F32, tag="ps")
                    psB = psL1.tile([P, 512], F32, tag="ps")
                    nc.tensor.matmul(psA[:], WF, xb[:, 0:1024:2],
                                     start=True, stop=True)
                    nc.tensor.matmul(psB[:], WF, xb[:, 1:1024:2],
                                     start=True, stop=True)
                    ab = abpool.tile([P, 1024], F16, tag="ab")
                    nc.scalar.copy(ab[:, 0:512], psA[:])
                    nc.vector.tensor_copy(ab[:, 512:1024], psB[:])
                    o = 512 * u
                    eng = nc.gpsimd if u in (1, 4, 6) else nc.vector
                    eng.tensor_tensor(Ustg[:, o : o + 512],
                                      ab[:, 0:512], ab[:, 512:1024], AL.add)
                    eng.tensor_tensor(Vstg[:, o : o + 512],
                                      ab[:, 0:512], ab[:, 512:1024], AL.subtract)

            def l2_group(g, Ustg, b3all, VDtmp):
                """cA1 rows 128g..+127: 2 contiguous-rhs matmuls -> parity
                interleaved psum; strided combines place cH2 lane-aligned
                in b3all, cV2/cD2 in VDtmp."""
                q, s = g % 2, g // 2
                WA, WB = WL2[q]
                ps2 = psL2.tile([P, 512], F32, tag="l2")
                nc.tensor.matmul(ps2[:], WA, Ustg[:, 1024 * g : 1024 * g + 512],
                                 start=True, stop=False)
                nc.tensor.matmul(ps2[:], WB,
                                 Ustg[:, 1024 * g + 512 : 1024 * g + 1024],
                                 start=False, stop=True)
                a2 = l2pool.tile([P, 512], F16, tag="a2b2")
                nc.scalar.copy(a2[:], ps2[:])
                so, do = 64 * q, 64 * (1 - q)
                ae = a2[:, 0:512:2]
                ao = a2[:, 1:512:2]
                # cH2 = S2e - S2o, lane-aligned into b3all H block
                nc.vector.tensor_tensor(
                    b3all[so : so + 64, 768 * s : 768 * s + 256],
                    ae[so : so + 64, :], ao[so : so + 64, :], AL.subtract)
                # cV2 = D2e + D2o, cD2 = D2e - D2o (wrong half; shifted later)
                nc.vector.tensor_tensor(
                    VDtmp[do : do + 64, 512 * s + 256 * q :
                          512 * s + 256 * q + 256],
                    ae[do : do + 64, :], ao[do : do + 64, :], AL.add)
                nc.vector.tensor_tensor(
                    VDtmp[do : do + 64, 1024 + 512 * s + 256 * q :
                          1024 + 512 * s + 256 * q + 256],
                    ae[do : do + 64, :], ao[do : do + 64, :], AL.subtract)

            def wup_stage(b, b3all, VDtmp):
                """Shift cV2/cD2 into b3all, W-upsample b3all -> wall
                [128, (s)(band)(even256|odd256)] (parity-blocked)."""
                for q in (0, 1):
                    for bb in (0, 1):  # 0 = V, 1 = D
                        src = VDtmp[64 * (1 - q) : 64 * (2 - q),
                                    1024 * bb : 1024 * (bb + 1)].rearrange(
                            "p (s c) -> p s c", s=2)[:, :, 256 * q : 256 * q + 256]
                        dst = b3all[64 * q : 64 * q + 64, :].rearrange(
                            "p (s h c) -> p s h c", s=2, h=3)[:, :, bb + 1, :]
                        nc.sync.dma_start(dst, src)
                wall = wpool.tile([P, 3072], F16, tag="wall", name="wall")
                t3 = t3pool.tile([P, 1536], F16, tag="t3", name="t3")
                nc.vector.tensor_scalar_mul(t3[:], b3all[:], 3.0)
                s4 = b3all[:].rearrange("p (s h c) -> p s h c", s=2, h=3)
                t4 = t3[:].rearrange("p (s h c) -> p s h c", s=2, h=3)
                d4 = wall[:].rearrange("p (s h x) -> p s h x", s=2, h=3)
                # even block: wu[2c] = 3b[c] + b[c-1]; odd: wu[2c+1] = 3b[c] + b[c+1]
                nc.vector.tensor_tensor(
                    d4[:, :, :, 1:256], t4[:, :, :, 1:256],
                    s4[:, :, :, 0:255], AL.add)
                nc.vector.tensor_tensor(
                    d4[:, :, :, 256:511], t4[:, :, :, 0:255],
                    s4[:, :, :, 1:256], AL.add)
                nc.vector.tensor_scalar_mul(
                    d4[:, :, :, 0:512:511], s4[:, :, :, 0:256:255], 4.0)
                return wall

            def evac_up(st, j, src_ap, k):
                # interleave even|odd parity blocks while evacuating
                dst = st[:, 512 * j : 512 * j + 512].rearrange(
                    "p (c par) -> p par c", par=2)
                src = src_ap.rearrange("p (par c) -> p par c", par=2)
                if k % 3 == 2:
                    nc.vector.tensor_copy(dst, src)
                else:
                    nc.scalar.copy(dst, src)

            def stage_b1(b, wall, sts):
                """H-up blocks 0 and 3 + halo row swaps for image b."""
                k = 0
                for j, Uw, wo in ((0, U0, 0), (3, U3, 1536)):
                    for band in range(3):
                        if j == 0:
                            st = stpool.tile([P, 2048], F16,
                                             tag=f"s2b{band}", name=f"s2b{band}")
                            sts.append(st)
                        else:
                            st = sts[band]
                        up = psUp.tile([P, 512], F32, tag="up")
                        nc.tensor.matmul(
                            up[:], Uw, wall[:, wo + 512 * band : wo + 512 * (band + 1)],
                            start=True, stop=True)
                        evac_up(st, j, up[:], k)
                        k += 1
                # halo: w0 row0 <- w1 row0; w1 row127 <- w0 row127
                nc.sync.dma_start(wall[0:1, 0:1536], wall[0:1, 1536:3072])
                nc.sync.dma_start(wall[127:128, 1536:3072], wall[127:128, 0:1536])

            def stage_b2(b, wall, sts):
                """H-up blocks 1 and 2 (halo'd) + output DMA for image b."""
                k = 3
                for j, Uw, wo in ((1, U1p, 0), (2, U2p, 1536)):
                    for band in range(3):
                        up = psUp.tile([P, 512], F32, tag="up")
                        nc.tensor.matmul(
                            up[:], Uw, wall[:, wo + 512 * band : wo + 512 * (band + 1)],
                            start=True, stop=True)
                        evac_up(sts[band], j, up[:], k)
                        k += 1
                for band in range(3):
                    dst = y_d[b, 3 + band]
                    nc.gpsimd.dma_start(
                        dst.rearrange("(u p) w -> p u w", u=4),
                        sts[band][:].rearrange("p (u w) -> p u w", u=4))

            pending = None
            for b in range(IMG):
                Ustg = uvpool.tile([P, 4096], F16, tag="U", name="Ustg")
                Vstg = uvpool.tile([P, 4096], F16, tag="V", name="Vstg")
                b3all = b3pool.tile([P, 1536], F16, tag="b3", name="b3all")
                VDtmp = vdpool.tile([P, 2048], F16, tag="vd", name="VDtmp")
                l1_half(b, 0, Ustg, Vstg)
                if pending is not None:
                    stage_b1(pending[0], pending[1], pending[2])
                l1_half(b, 1, Ustg, Vstg)
                for g in range(4):
                    l2_group(g, Ustg, b3all, VDtmp)
                # L1 band outputs: cH1=V[0:64], cV1=U[64:128], cD1=V[64:128]
                for band, (stg, lo) in enumerate(
                        ((Vstg, 0), (Ustg, 64), (Vstg, 64))):
                    src = stg[lo : lo + 64, :].rearrange(
                        "p (u w) -> p u w", u=8)
                    nc.gpsimd.dma_start(
                        y_d[b, band].rearrange("(u p) w -> p u w", u=8), src)
                wall = wup_stage(b, b3all, VDtmp)
                if pending is not None:
                    stage_b2(pending[0], pending[1], pending[2])
                pending = (b, wall, [])
            stage_b1(pending[0], pending[1], pending[2])
            stage_b2(pending[0], pending[1], pending[2])

    nc.compile()
    return nc


_NC_CACHE = None
LAST_RESULTS = None


def kernel(**inputs) -> np.ndarray:
    global _NC_CACHE, LAST_RESULTS
    trace = bool(inputs.pop("_trace", False))
    x = np.asarray(inputs["x"])
    assert x.shape == (B, 1, H, W), x.shape
    x16 = np.ascontiguousarray(x.astype(np.float16))
    if _NC_CACHE is None:
        _NC_CACHE = build_nc()
    nc = _NC_CACHE
    w16 = _build_w16()
    in_maps = [
        {"xc": np.ascontiguousarray(x16[IMG * c : IMG * (c + 1), 0]), "w16": w16}
        for c in range(NCORES)
    ]
    res = bass_utils.run_bass_kernel_spmd(
        nc, in_maps, core_ids=list(range(NCORES)), trace=trace
    )
    LAST_RESULTS = res
    out = np.concatenate([res.results[c]["yc"] for c in range(NCORES)], axis=0)
    return out.astype(np.float32)


if __name__ == "__main__":
    rng = np.random.default_rng(0)
    x = rng.standard_normal((B, 1, H, W), dtype=np.float32)
    y = kernel(x=x)
    print("kernel output:", y.shape, y.dtype)


# revision 17
# speedup vs baseline: 1.1303x; 1.1303x over previous
"""Trainium2 Bass kernel for nn_DWTExtractor: 2-level Haar DWT + bilinear 2x upsample.

Input  x: (32, 1, 1024, 1024) fp32
Output y: (32, 6, 512, 512) fp32 = [cH1, cV1, cD1, cH2u, cV2u, cD2u]

Sharding: pure batch data-parallel, 4 images per core across 8 cores.

v9 design (= v4 "all combines folded into PSUM accumulation" + finer
input loads + last-image L2 interleave). Measured orderings: folding
band combines into extra accumulating matmuls beats vector-engine
combines (v8 regression); interleaving L2 into every image's L1 stream
hurts (v5/v6 regressions) except for the last image where it shortens
the serial drain tail.
  - fp16 datapath (host converts, ~1e-3 rel err).
  - L1 Haar per 128-row block: U = [cA1|cV1] and V = [cH1|cD1] each via
    TWO accumulating matmuls (fused sum/diff weight WF on even cols,
    +-WF on odd cols). No vector combines; evacuation is a pure f32->f16
    copy, alternating ACT (U) / DVE (V).
  - L2 same trick on cA1 (Ustg parts 0..63, zero-padded weights): per-g
    psum HD = [cH2 | cD2-wrong-half], psum V = [cA2junk | cV2]. cH2
    copies lane-aligned into row-major b3all (per-parity weight variants
    put S2 at partitions 64(g%2)); cV2/cD2 to VDtmp + 4 shift DMAs.
  - W-upsample: t3 = 3*b3 (tensor_scalar 4x) + two shifted adds (2x)
    into parity-BLOCKED wall; e/o interleave deferred to H-up evac APs.
  - H-upsample: 12 matmuls + halo row swap.
  - Hazards: interleaved PSUM groups must sit in different banks; DMA
    dst APs need one uniform partition shift; >3-dim DMA APs fail.
  - Triggers: Sync = input only, GPSIMD = shifts/halo/band outputs.
"""

import numpy as np

import concourse.bass as bass
import concourse.tile as tile
import concourse.mybir as mybir
from concourse import bacc, bass_utils

F32 = mybir.dt.float32
F16 = mybir.dt.float16
AL = mybir.AluOpType

B, H, W = 32, 1024, 1024
NCORES = 8
IMG = B // NCORES  # images per core
HL, WL = H // 2, W // 2  # 512 (level-1 band size)
H2, W2 = H // 4, W // 4  # 256 (level-2 band size)
P = 128


def _build_w16() -> np.ndarray:
    """(128, 14*128) fp16:
    WF | WFN | A0 B0 An0 Bn0 | A1 B1 An1 Bn1 | U0 U1p U2p U3.

    WF: out parts [row-pair sums | row-pair diffs].  A/B (parity q):
    S2-pairs at out parts 64q+i, D2-pairs at 64(1-q)+i; An/Bn negated.
    """
    wf = np.zeros((P, P), np.float16)
    for i in range(64):
        wf[2 * i, i] = 0.5
        wf[2 * i + 1, i] = 0.5
        wf[2 * i, 64 + i] = 0.5
        wf[2 * i + 1, 64 + i] = -0.5

    wl2 = []
    for q in (0, 1):
        a = np.zeros((P, P), np.float16)
        bq = np.zeros((P, P), np.float16)
        so, do = 64 * q, 64 * (1 - q)
        for i in range(32):
            a[2 * i, so + i] = 0.5
            a[2 * i + 1, so + i] = 0.5
            a[2 * i, do + i] = 0.5
            a[2 * i + 1, do + i] = -0.5
            bq[2 * i, so + 32 + i] = 0.5
            bq[2 * i + 1, so + 32 + i] = 0.5
            bq[2 * i, do + 32 + i] = 0.5
            bq[2 * i + 1, do + 32 + i] = -0.5
        wl2 += [a, bq, -a, -bq]

    u_full = np.zeros((H2, HL), np.float32)
    for m in range(HL):
        k = m // 2
        taps = [(k, 0.75), (k - 1, 0.25)] if m % 2 == 0 else [(k, 0.75), (k + 1, 0.25)]
        for src, wgt in taps:
            u_full[min(max(src, 0), H2 - 1), m] += wgt
    u_full *= 0.25
    u0 = u_full[0:128, 0:128].astype(np.float16)
    u1p = u_full[0:128, 128:256].astype(np.float16)
    u1p[0, :] = u_full[128, 128:256].astype(np.float16)  # halo tap row
    u2p = u_full[128:256, 256:384].astype(np.float16)
    u2p[127, :] = u_full[127, 256:384].astype(np.float16)  # halo tap row
    u3 = u_full[128:256, 384:512].astype(np.float16)

    return np.concatenate([wf, -wf] + wl2 + [u0, u1p, u2p, u3], axis=1)


def build_nc() -> "bacc.Bacc":
    nc = bacc.Bacc(
        "TRN2", target_bir_lowering=False, debug=False, num_devices=NCORES,
        name="dwt_extractor",
    )
    x_d = nc.dram_tensor("xc", [IMG, H, W], F16, kind="ExternalInput")
    w16_d = nc.dram_tensor("w16", [P, 14 * P], F16, kind="ExternalInput")
    y_d = nc.dram_tensor("yc", [IMG, 6, HL, WL], F16, kind="ExternalOutput")

    with tile.TileContext(nc) as tc:
        with (
            tc.tile_pool(name="consts", bufs=1) as cpool,
            tc.tile_pool(name="xin", bufs=6) as xpool,
            tc.tile_pool(name="uv", bufs=2) as uvpool,
            tc.tile_pool(name="vdt", bufs=2) as vdpool,
            tc.tile_pool(name="b3", bufs=2) as b3pool,
            tc.tile_pool(name="t3p", bufs=2) as t3pool,
            tc.tile_pool(name="wtile", bufs=2) as wpool,
            tc.tile_pool(name="stg2", bufs=2) as stpool,
            tc.tile_pool(name="psL1", bufs=4, space="PSUM") as psL1,
            tc.tile_pool(name="psL2", bufs=2, space="PSUM") as psL2,
            tc.tile_pool(name="psUp", bufs=2, space="PSUM") as psUp,
        ):
            w16 = cpool.tile([P, 14 * P], F16)
            nc.sync.dma_start(w16[:], w16_d[:])
            blk = lambda i: w16[:, i * P : (i + 1) * P]
            WF, WFN = blk(0), blk(1)
            WL2 = [(blk(2), blk(3), blk(4), blk(5)),
                   (blk(6), blk(7), blk(8), blk(9))]  # [q] -> (A, B, An, Bn)
            U0, U1p, U2p, U3 = blk(10), blk(11), blk(12), blk(13)

            def l1_pair(b, up_, Ustg, Vstg):
                """Two 128-row blocks: one load; per block U/V built by
                accumulating matmuls, evac copy ACT (U) / DVE (V)."""
                xu = xpool.tile([P, 2048], F16, tag="x")
                src = x_d[b, 256 * up_ : 256 * (up_ + 1), :]
                nc.sync.dma_start(
                    xu[:].rearrange("p (t w) -> p t w", t=2),
                    src.rearrange("(t p) w -> p t w", t=2),
                )
                for t in range(2):
                    u = 2 * up_ + t
                    xb = xu[:, 1024 * t : 1024 * (t + 1)]
                    xe, xo = xb[:, 0:1024:2], xb[:, 1:1024:2]
                    psU = psL1.tile([P, 512], F32, tag="ps")
                    psV = psL1.tile([P, 512], F32, tag="ps")
                    nc.tensor.matmul(psU[:], WF, xe, start=True, stop=False)
                    nc.tensor.matmul(psV[:], WF, xe, start=True, stop=False)
                    nc.tensor.matmul(psU[:], WF, xo, start=False, stop=True)
                    nc.tensor.matmul(psV[:], WFN, xo, start=False, stop=True)
                    o = 512 * u
                    nc.scalar.copy(Ustg[:, o : o + 512], psU[:])
                    nc.vector.tensor_copy(Vstg[:, o : o + 512], psV[:])

            def l2_group(g, Ustg, b3all, VDtmp):
                """cA1 rows 128g..+127 -> psum HD = [cH2 | cD2'], psum V =
                [cA2junk | cV2']; copies go lane-aligned / to VDtmp."""
                q, s = g % 2, g // 2
                WA, WB, WAn, WBn = WL2[q]
                ue0 = Ustg[:, 1024 * g : 1024 * g + 512]
                ue1 = Ustg[:, 1024 * g + 512 : 1024 * g + 1024]
                e0, o0 = ue0[:, 0:512:2], ue0[:, 1:512:2]
                e1, o1 = ue1[:, 0:512:2], ue1[:, 1:512:2]
                psHD = psL2.tile([P, 256], F32, tag="ps2", padded_shape=[P, 512])
                psV = psL2.tile([P, 256], F32, tag="ps2", padded_shape=[P, 512])
                # grouped by weight; HD and V groups sit in different banks
                nc.tensor.matmul(psHD[:], WA, e0, start=True, stop=False)
                nc.tensor.matmul(psV[:], WA, e0, start=True, stop=False)
                nc.tensor.matmul(psV[:], WA, o0, start=False, stop=False)
                nc.tensor.matmul(psHD[:], WB, e1, start=False, stop=False)
                nc.tensor.matmul(psV[:], WB, e1, start=False, stop=False)
                nc.tensor.matmul(psV[:], WB, o1, start=False, stop=True)
                nc.tensor.matmul(psHD[:], WAn, o0, start=False, stop=False)
                nc.tensor.matmul(psHD[:], WBn, o1, start=False, stop=True)
                so, do = 64 * q, 64 * (1 - q)
                # cH2 lane-aligned into b3all H block
                nc.scalar.copy(
                    b3all[so : so + 64, 768 * s : 768 * s + 256],
                    psHD[so : so + 64, :])
                # cD2 / cV2 on the wrong half -> VDtmp (shifted later)
                nc.vector.tensor_copy(
                    VDtmp[do : do + 64, 1024 + 512 * s + 256 * q :
                          1024 + 512 * s + 256 * q + 256],
                    psHD[do : do + 64, :])
                nc.vector.tensor_copy(
                    VDtmp[do : do + 64, 512 * s + 256 * q :
                          512 * s + 256 * q + 256],
                    psV[do : do + 64, :])

            def wup_stage(b, b3all, VDtmp):
                """Shift cV2/cD2 into b3all, W-upsample b3all -> wall
                [128, (s)(band)(even256|odd256)] (parity-blocked)."""
                for q in (0, 1):
                    for bb in (0, 1):  # 0 = V, 1 = D
                        src = VDtmp[64 * (1 - q) : 64 * (2 - q),
                                    1024 * bb : 1024 * (bb + 1)].rearrange(
                            "p (s c) -> p s c", s=2)[:, :, 256 * q : 256 * q + 256]
                        dst = b3all[64 * q : 64 * q + 64, :].rearrange(
                            "p (s h c) -> p s h c", s=2, h=3)[:, :, bb + 1, :]
                        nc.gpsimd.dma_start(dst, src)
                wall = wpool.tile([P, 3072], F16, tag="wall", name="wall")
                t3 = t3pool.tile([P, 1536], F16, tag="t3", name="t3")
                nc.vector.tensor_scalar_mul(t3[:], b3all[:], 3.0)
                s4 = b3all[:].rearrange("p (s h c) -> p s h c", s=2, h=3)
                t4 = t3[:].rearrange("p (s h c) -> p s h c", s=2, h=3)
                d4 = wall[:].rearrange("p (s h x) -> p s h x", s=2, h=3)
                # even block: wu[2c] = 3b[c] + b[c-1]; odd: wu[2c+1] = 3b[c] + b[c+1]
                nc.vector.tensor_tensor(
                    d4[:, :, :, 1:256], t4[:, :, :, 1:256],
                    s4[:, :, :, 0:255], AL.add)
                nc.vector.tensor_tensor(
                    d4[:, :, :, 256:511], t4[:, :, :, 0:255],
                    s4[:, :, :, 1:256], AL.add)
                nc.vector.tensor_scalar_mul(
                    d4[:, :, :, 0:512:511], s4[:, :, :, 0:256:255], 4.0)
                return wall

            def evac_up(st, j, src_ap, k):
                # interleave even|odd parity blocks while evacuating
                dst = st[:, 512 * j : 512 * j + 512].rearrange(
                    "p (c par) -> p par c", par=2)
                src = src_ap.rearrange("p (par c) -> p par c", par=2)
                if k % 3 == 2:
                    nc.vector.tensor_copy(dst, src)
                else:
                    nc.scalar.copy(dst, src)

            def stage_b1(b, wall, sts):
                """H-up blocks 0 and 3 + halo row swaps for image b."""
                k = 0
                for j, Uw, wo in ((0, U0, 0), (3, U3, 1536)):
                    for band in range(3):
                        if j == 0:
                            st = stpool.tile([P, 2048], F16,
                                             tag=f"s2b{band}", name=f"s2b{band}")
                            sts.append(st)
                        else:
                            st = sts[band]
                        up = psUp.tile([P, 512], F32, tag="up")
                        nc.tensor.matmul(
                            up[:], Uw, wall[:, wo + 512 * band : wo + 512 * (band + 1)],
                            start=True, stop=True)
                        evac_up(st, j, up[:], k)
                        k += 1
                # halo: w0 row0 <- w1 row0; w1 row127 <- w0 row127
                nc.gpsimd.dma_start(wall[0:1, 0:1536], wall[0:1, 1536:3072])
                nc.gpsimd.dma_start(wall[127:128, 1536:3072], wall[127:128, 0:1536])

            def stage_b2(b, wall, sts):
                """H-up blocks 1 and 2 (halo'd) + output DMA for image b."""
                k = 3
                for j, Uw, wo in ((1, U1p, 0), (2, U2p, 1536)):
                    for band in range(3):
                        up = psUp.tile([P, 512], F32, tag="up")
                        nc.tensor.matmul(
                            up[:], Uw, wall[:, wo + 512 * band : wo + 512 * (band + 1)],
                            start=True, stop=True)
                        evac_up(sts[band], j, up[:], k)
                        k += 1
                for band in range(3):
                    dst = y_d[b, 3 + band]
                    nc.gpsimd.dma_start(
                        dst.rearrange("(u p) w -> p u w", u=4),
                        sts[band][:].rearrange("p (u w) -> p u w", u=4))

            pending = None
            for b in range(IMG):
                last = b == IMG - 1
                Ustg = uvpool.tile([P, 4096], F16, tag="U", name="Ustg")
                Vstg = uvpool.tile([P, 4096], F16, tag="V", name="Vstg")
                b3all = b3pool.tile([P, 1536], F16, tag="b3", name="b3all")
                VDtmp = vdpool.tile([P, 2048], F16, tag="vd", name="VDtmp")
                l1_pair(b, 0, Ustg, Vstg)
                l1_pair(b, 1, Ustg, Vstg)
                if last:
                    l2_group(0, Ustg, b3all, VDtmp)
                    l2_group(1, Ustg, b3all, VDtmp)
                if pending is not None:
                    stage_b1(pending[0], pending[1], pending[2])
                l1_pair(b, 2, Ustg, Vstg)
                l1_pair(b, 3, Ustg, Vstg)
                for g in ((2, 3) if last else (0, 1, 2, 3)):
                    l2_group(g, Ustg, b3all, VDtmp)
                # L1 band outputs: cH1=V[0:64], cV1=U[64:128], cD1=V[64:128]
                for band, (stg, lo) in enumerate(
                        ((Vstg, 0), (Ustg, 64), (Vstg, 64))):
                    src = stg[lo : lo + 64, :].rearrange(
                        "p (u w) -> p u w", u=8)
                    nc.gpsimd.dma_start(
                        y_d[b, band].rearrange("(u p) w -> p u w", u=8), src)
                wall = wup_stage(b, b3all, VDtmp)
                if pending is not None:
                    stage_b2(pending[0], pending[1], pending[2])
                pending = (b, wall, [])
            stage_b1(pending[0], pending[1], pending[2])
            stage_b2(pending[0], pending[1], pending[2])

    nc.compile()
    return nc


_NC_CACHE = None
LAST_RESULTS = None


def kernel(**inputs) -> np.ndarray:
    global _NC_CACHE, LAST_RESULTS
    trace = bool(inputs.pop("_trace", False))
    x = np.asarray(inputs["x"])
    assert x.shape == (B, 1, H, W), x.shape
    x16 = np.ascontiguousarray(x.astype(np.float16))
    if _NC_CACHE is None:
        _NC_CACHE = build_nc()
    nc = _NC_CACHE
    w16 = _build_w16()
    in_maps = [
        {"xc": np.ascontiguousarray(x16[IMG * c : IMG * (c + 1), 0]), "w16": w16}
        for c in range(NCORES)
    ]
    res = bass_utils.run_bass_kernel_spmd(
        nc, in_maps, core_ids=list(range(NCORES)), trace=trace
    )
    LAST_RESULTS = res
    out = np.concatenate([res.results[c]["yc"] for c in range(NCORES)], axis=0)
    return out.astype(np.float32)


if __name__ == "__main__":
    rng = np.random.default_rng(0)
    x = rng.standard_normal((B, 1, H, W), dtype=np.float32)
    y = kernel(x=x)
    print("kernel output:", y.shape, y.dtype)
